# revision 1
# baseline (speedup 1.0000x reference)
"""Two-layer GAT on 8 trn2 NeuronCores.

Design (slot layout):
- Nodes are sorted by in-degree and grouped into 392 blocks of 128; blocks are
  dealt round-robin to the 8 cores (49 blocks/core). One dst node per SBUF
  partition; its incoming edges occupy free-dim "slots". Block tile count
  J_b = max in-block degree (tight because blocks are degree-sorted).
- Per-edge source rows are fetched with InstDMAGatherAnt from a replicated
  feature table, int16 indices biased by -32768 against a table AP sliced at
  row 32768 (the Q7 ucode multiplies SIGNED idx x stride, so negative indices
  address backward — covers all 50176 rows).
- Attention: e = leaky(el_src + er_dst); er_dst is a stride-0 AP broadcast
  (dst == partition). Pad slots get -10000 added before exp -> weight 0.
- Aggregation: per-tile matmul with a constant identity lhsT accumulating in
  PSUM == per-partition sum over slots of eexp-scaled messages. Softmax
  denominator is a strided DVE reduce of the eexp column; 1/denom is applied
  post-aggregation (softmax scale-invariance).
- Layer boundary: each core computes feat2 = h @ W2ext for its own nodes and
  the [50176]-row layer-2 table is rebuilt via an 8-core AllGather. Same for
  the layer-1 table after the feat1 matmuls.
"""

import os
import math
import numpy as np
from contextlib import ExitStack

import concourse.bass as bass
import concourse.tile as tile
from concourse import bacc, mybir
from concourse.bass_utils import run_bass_kernel_spmd
from concourse.masks import make_identity

P = 128
NCORES = 8
N = 50000
E = 800000
IN_F = 256
H1, D1 = 4, 64
HID = H1 * D1          # 256
OUT_F = 40
NEG_SLOPE = 0.2

NPAD = 50176           # 392 blocks * 128
NBLK_TOTAL = NPAD // P  # 392
NBLK = NBLK_TOTAL // NCORES  # 49 per core
SHARD = NBLK * P       # 6272 rows per core shard

ROW1 = 384             # fp16 elems per L1 table row (768B stride); data in 0:260
ELEM1 = 260            # gathered fp16 elems (feat1 256 + el1 4) = 520B
ROW2 = 128             # fp16 elems per L2 table row (256B stride); data in 0:41
ELEM2 = 42             # gathered fp16 elems (feat2 40 + el2 1 + 1 garbage) = 84B
SHIFT = 32768
PAD_ROW = 40000        # full-table row used by pad slots (any valid row)
EADD_PAD = -10000.0
EXP_BIAS = -3.0        # eexp = exp(e - 3): fp16-safe for e in [-5, 11]
CHUNK = 16             # max tiles per gather call (ring limit ~2300 idx works,
                       # stay at 16*128+16 = 2064 for margin)


def _pool_gather(nc, out_ap, in_ap, idxs_ap, num_idxs, elem_size, queue_num=0):
    """InstDMAGatherAnt without the %256 elem-size assert, single_packet=False."""
    g = nc.gpsimd
    elem_step = in_ap.ap[0][0]
    stride_bytes = elem_step * mybir.dt.size(in_ap.dtype)
    stride_bytes_256 = stride_bytes // 256
    assert stride_bytes % 256 == 0 and stride_bytes_256 < 256, stride_bytes
    _in_ap = g.lower_ap_dma(in_ap, for_custom_bir_dma=True)
    _idxs_ap = g.lower_ap(idxs_ap)
    _out_ap = g.lower_ap(out_ap)
    return g.add_instruction(
        mybir.InstDMAGatherAnt(
            name=nc.get_next_instruction_name(),
            ins=[*_in_ap, _idxs_ap, g.lower_val_access(g.to_reg(num_idxs))],
            outs=[_out_ap],
            transpose=False,
            num_idxs=num_idxs,
            elem_size=elem_size,
            stride_bytes_256=stride_bytes_256,
            gen_mode=0,
            single_packet=False,
            queue_num=queue_num,
        )
    )


def build_plan(src, dst):
    """Host-side graph preprocessing.

    Returns (merged_layout, per_core_streams, meta):
      merged_layout: calls/Jb/jb_off/T/idx_tile_cols shared by every core
      per_core_streams: idx_tile [P, cols] int16, eadd_tile [P, T] f32
    """
    cnt = np.bincount(dst, minlength=N)
    order = np.argsort(cnt, kind="stable")          # ascending in-degree
    pos_of_node = np.empty(N, dtype=np.int64)
    pos_of_node[order] = np.arange(N)

    gblk_arr = np.arange(NPAD) // P
    row_of_pos = (gblk_arr % NCORES) * SHARD + (gblk_arr // NCORES) * P + (np.arange(NPAD) % P)
    row_of_node = row_of_pos[pos_of_node]           # [N]

    e_pos = pos_of_node[dst]
    e_gblk = e_pos // P
    e_core = e_gblk % NCORES
    e_lblk = e_gblk // NCORES
    e_p = e_pos % P
    e_row = row_of_node[src]

    # per-core slot arrays: idx_slot[c][b] : [Jb, P] table-row ids (PAD_ROW pads)
    deg = np.zeros((NCORES, NBLK, P), dtype=np.int64)
    np.add.at(deg, (e_core, e_lblk, e_p), 1)
    Jb = np.maximum(deg.max(axis=(0, 2)), 1)        # merged per-block tile count
    Jmax = int(Jb.max())

    idx_slot = np.full((NCORES, NBLK, Jmax, P), PAD_ROW, dtype=np.int64)
    eadd_slot = np.full((NCORES, NBLK, Jmax, P), EADD_PAD, dtype=np.float32)
    # slot j for each edge: order within its (core, block, partition) group
    key = (e_core.astype(np.int64) * NBLK + e_lblk) * P + e_p
    sort = np.argsort(key, kind="stable")
    ks, rs = key[sort], e_row[sort]
    first = np.r_[True, ks[1:] != ks[:-1]]
    grp_start = np.flatnonzero(first)
    grp_len = np.diff(np.r_[grp_start, len(ks)])
    j_in_grp = np.arange(len(ks)) - np.repeat(grp_start, grp_len)
    cs, bs, ps_ = e_core[sort], e_lblk[sort], e_p[sort]
    idx_slot[cs, bs, j_in_grp, ps_] = rs
    eadd_slot[cs, bs, j_in_grp, ps_] = 0.0

    # shared call layout
    calls = []
    col0 = 0
    for b in range(NBLK):
        for j0 in range(0, int(Jb[b]), CHUNK):
            span = min(CHUNK, int(Jb[b]) - j0)
            nidx = span * P + 16
            calls.append((b, j0, span, col0, nidx // 16, nidx))
            col0 += nidx // 16
    jb_off = np.concatenate([[0], np.cumsum(Jb)]).astype(np.int64)
    merged = dict(calls=calls, Jb=Jb.astype(np.int64), jb_off=jb_off,
                  T=int(Jb.sum()), idx_tile_cols=col0, Jmax=Jmax)

    streams = []
    armod = np.arange(P) % 16
    for c in range(NCORES):
        idx_tile = np.zeros((P, col0), dtype=np.int16)
        for (b, j0, span, c0, ncols, nidx) in calls:
            flat = np.concatenate([
                idx_slot[c, b, j0:j0 + span].reshape(-1),
                np.full(16, PAD_ROW, dtype=np.int64),
            ])
            i16 = (flat - SHIFT).astype(np.int16)
            # wrapped-16 layout: column k holds idx[k*16 + (p % 16)]
            idx_tile[:, c0:c0 + ncols] = i16.reshape(ncols, 16)[:, armod].T
        eadd_tile = np.concatenate(
            [eadd_slot[c, b, 0:int(Jb[b])].T for b in range(NBLK)], axis=1
        ).astype(np.float32)
        streams.append(dict(idx_tile=idx_tile, eadd_tile=eadd_tile))
    meta = dict(order=order)
    return merged, streams, meta


def build_nc(plan, Jmax, reps=1, single=False):
    """Build the Bass program (same program for all cores; plan gives shapes).

    All cores share one SPMD program, so the plan used for instruction layout
    must be identical across cores -> we require identical calls/Jb across
    cores (guaranteed by padding to common Jb per (core,block)? NO -- instead
    the caller passes the per-core MERGED plan: calls/Jb are the elementwise
    max across cores, and per-core idx/eadd streams are padded to that shape).
    """
    nc = bacc.Bacc("TRN2", target_bir_lowering=False, debug=False,
                   enable_asserts=False, num_devices=(1 if single else NCORES))
    dt = mybir.dt
    # register the exp bias as a const AP (only 0.0/1.0 exist by default)
    _cb = nc.alloc_sbuf_tensor(f"const-float32-expbias", [128, 1], dt.float32)
    nc.gpsimd.memset(_cb.ap(), EXP_BIAS)
    nc.const_aps.aps[(dt.float32, EXP_BIAS)] = _cb.ap()
    nc.all_engine_barrier()
    calls = plan["calls"]
    Jb = plan["Jb"]
    jb_off = plan["jb_off"]
    T = plan["T"]
    ncols_total = plan["idx_tile_cols"]

    # ---- external I/O ----
    xT_d = nc.dram_tensor("xT", [IN_F, SHARD], dt.float16, kind="ExternalInput")
    w1e_d = nc.dram_tensor("w1e", [IN_F, HID + 8], dt.float16, kind="ExternalInput")
    w2e_d = nc.dram_tensor("w2e", [HID, OUT_F + 2], dt.float16, kind="ExternalInput")
    idx_d = nc.dram_tensor("idx", [P, ncols_total], dt.int16, kind="ExternalInput")
    eadd_d = nc.dram_tensor("eadd", [P, T], dt.float32, kind="ExternalInput")
    out_d = nc.dram_tensor("logits", [SHARD, OUT_F], dt.float32, kind="ExternalOutput")

    # ---- internal DRAM ----
    t1_shard = nc.dram_tensor("t1_shard", [SHARD, ROW1], dt.float16)
    t1_full = nc.dram_tensor("t1_full", [NPAD, ROW1], dt.float16, addr_space="Shared")
    t2_shard = nc.dram_tensor("t2_shard", [SHARD, ROW2], dt.float16)
    t2_full = nc.dram_tensor("t2_full", [NPAD, ROW2], dt.float16, addr_space="Shared")

    groups = [list(range(NCORES))]

    with tile.TileContext(nc) as tc, ExitStack() as ctx:
        const = ctx.enter_context(tc.tile_pool(name="const", bufs=1))
        sb = ctx.enter_context(tc.tile_pool(name="sb", bufs=2))
        gpool = ctx.enter_context(tc.tile_pool(name="gath", bufs=3))
        epool = ctx.enter_context(tc.tile_pool(name="edge", bufs=3))
        hpool = ctx.enter_context(tc.tile_pool(name="hid", bufs=3))
        ps = ctx.enter_context(tc.tile_pool(name="ps", bufs=2, space="PSUM"))
        pst = ctx.enter_context(tc.tile_pool(name="pst", bufs=2, space="PSUM"))

        # constants
        ident = const.tile([P, P], dt.float16)
        make_identity(nc, ident[:])
        w1e = const.tile([P, 2, HID + 8], dt.float16)
        nc.sync.dma_start(out=w1e[:], in_=w1e_d[:, :].rearrange("(k f) n -> f k n", k=2))
        w2e = const.tile([P, 2, OUT_F + 2], dt.float16)
        nc.sync.dma_start(out=w2e[:], in_=w2e_d[:, :].rearrange("(k f) n -> f k n", k=2))
        idx_sb = const.tile([P, ncols_total], dt.int16)
        nc.sync.dma_start(out=idx_sb[:], in_=idx_d[:, :])
        eadd_sb = const.tile([P, T], dt.float32)
        nc.sync.dma_start(out=eadd_sb[:], in_=eadd_d[:, :])
        er1_res = const.tile([P, NBLK * H1], dt.float16)
        er2_res = const.tile([P, NBLK], dt.float16)

        # zero gather buffers once (avoid NaN garbage on first partial use)
        gz = []
        for i in range(3):
            g1 = gpool.tile([P, Jmax + 1, ELEM1], dt.float16, tag="g1")
            nc.vector.memset(g1[:], 0.0)
            gz.append(g1)
        del gz

        for rep in range(reps):
            # ================= feat1 phase =================
            for b in range(NBLK):
                xt0 = sb.tile([P, P], dt.float16, tag="xt0")
                nc.sync.dma_start(out=xt0[:], in_=xT_d[0:P, b * P:(b + 1) * P])
                xt1 = sb.tile([P, P], dt.float16, tag="xt1")
                nc.sync.dma_start(out=xt1[:], in_=xT_d[P:2 * P, b * P:(b + 1) * P])
                pf = ps.tile([P, HID + 8], dt.float32, space="PSUM", tag="acc")
                nc.tensor.matmul(out=pf[:], lhsT=xt0[:], rhs=w1e[:, 0, :], start=True, stop=False)
                nc.tensor.matmul(out=pf[:], lhsT=xt1[:], rhs=w1e[:, 1, :], start=False, stop=True)
                # table row: [feat1 fp16 256 | el1 fp16 4] ; er1 -> resident
                trow = sb.tile([P, ELEM1], dt.float16, tag="trow")
                nc.vector.tensor_copy(trow[:, 0:HID + 4], pf[:, 0:HID + 4])
                nc.vector.tensor_copy(er1_res[:, b * H1:(b + 1) * H1], pf[:, HID + 4:HID + 8])
                nc.sync.dma_start(out=t1_shard[b * P:(b + 1) * P, 0:ELEM1], in_=trow[:])

            # ================= allgather L1 table =================
            if single:
                for c in range(NCORES):
                    nc.sync.dma_start(out=t1_full[c * SHARD:(c + 1) * SHARD, :], in_=t1_shard[:, :])
            else:
                nc.gpsimd.collective_compute(
                    "AllGather", mybir.AluOpType.bypass, replica_groups=groups,
                    ins=[t1_shard[:, :]], outs=[t1_full[:, :]],
                )

            # ================= layer 1 edge loop =================
            for b in range(NBLK):
                J = int(Jb[b])
                g1 = gpool.tile([P, Jmax + 1, ELEM1], dt.float16, tag="g1")
                for (cb, j0, span, col0, ncols, nidx) in calls:
                    if cb != b:
                        continue
                    _pool_gather(
                        nc, g1[:, j0:j0 + span + 1, :], t1_full[SHIFT:, :],
                        idx_sb[:, col0:col0 + ncols], nidx, ELEM1,
                    )
                el = g1[:, 0:J, HID:HID + 4]
                er_b = er1_res[:, b * H1:(b + 1) * H1].unsqueeze(1).to_broadcast([P, J, H1])
                e1 = epool.tile([P, Jmax, H1], dt.float32, tag="e1")
                nc.vector.tensor_tensor(out=e1[:, 0:J, :], in0=el, in1=er_b, op=mybir.AluOpType.add)
                e2 = epool.tile([P, Jmax, H1], dt.float32, tag="e2")
                nc.vector.tensor_scalar(out=e2[:, 0:J, :], in0=e1[:, 0:J, :], scalar1=NEG_SLOPE,
                                        scalar2=None, op0=mybir.AluOpType.mult)
                nc.vector.tensor_tensor(out=e2[:, 0:J, :], in0=e2[:, 0:J, :], in1=e1[:, 0:J, :],
                                        op=mybir.AluOpType.max)
                ea = eadd_sb[:, int(jb_off[b]):int(jb_off[b]) + J].unsqueeze(2).to_broadcast([P, J, H1])
                nc.vector.tensor_tensor(out=e2[:, 0:J, :], in0=e2[:, 0:J, :], in1=ea,
                                        op=mybir.AluOpType.add)
                # eexp (fp16) overwrites the el slot
                nc.scalar.activation(g1[:, 0:J, HID:HID + 4], e2[:, 0:J, :],
                                     mybir.ActivationFunctionType.Exp, bias=EXP_BIAS)
                # denominator: sum over slots per head
                den = epool.tile([P, H1], dt.float32, tag="den")
                nc.vector.tensor_reduce(
                    out=den[:], in_=g1[:, 0:J, HID:HID + 4].rearrange("p j h -> p h j"),
                    axis=mybir.AxisListType.X, op=mybir.AluOpType.add,
                )
                nc.vector.tensor_scalar(out=den[:], in0=den[:], scalar1=1e-30, scalar2=None,
                                        op0=mybir.AluOpType.add)
                rec = epool.tile([P, H1], dt.float32, tag="rec")
                nc.vector.reciprocal(rec[:], den[:])
                # scale messages by eexp (per head, stride-0 bcast over 64 dims)
                eexp_b = g1[:, 0:J, HID:HID + 4].unsqueeze(3).to_broadcast([P, J, H1, D1])
                nc.vector.tensor_tensor(
                    out=g1[:, 0:J, 0:HID].rearrange("p j (h d) -> p j h d", h=H1),
                    in0=g1[:, 0:J, 0:HID].rearrange("p j (h d) -> p j h d", h=H1),
                    in1=eexp_b, op=mybir.AluOpType.mult,
                )
                # aggregate over slots via identity matmuls
                pa = ps.tile([P, HID], dt.float32, space="PSUM", tag="acc")
                for j in range(J):
                    nc.tensor.matmul(out=pa[:], lhsT=ident[:], rhs=g1[:, j, 0:HID],
                                     start=(j == 0), stop=(j == J - 1))
                # normalize + ELU -> h (fp16)
                rec_b = rec[:].unsqueeze(2).to_broadcast([P, H1, D1])
                rstn = hpool.tile([P, HID], dt.float32, tag="rstn")
                nc.vector.tensor_tensor(out=rstn[:].rearrange("p (h d) -> p h d", h=H1),
                                        in0=pa[:].rearrange("p (h d) -> p h d", h=H1),
                                        in1=rec_b, op=mybir.AluOpType.mult)
                mn = hpool.tile([P, HID], dt.float32, tag="mn")
                nc.vector.tensor_scalar(out=mn[:], in0=rstn[:], scalar1=0.0, scalar2=None,
                                        op0=mybir.AluOpType.min)
                nc.scalar.activation(mn[:], mn[:], mybir.ActivationFunctionType.Exp)
                mx = hpool.tile([P, HID], dt.float32, tag="mx")
                nc.vector.tensor_scalar(out=mx[:], in0=rstn[:], scalar1=0.0, scalar2=None,
                                        op0=mybir.AluOpType.max)
                nc.vector.tensor_tensor(out=mn[:], in0=mn[:], in1=mx[:], op=mybir.AluOpType.add)
                h16 = hpool.tile([P, HID], dt.float16, tag="h16")
                nc.vector.tensor_scalar(out=h16[:], in0=mn[:], scalar1=-1.0, scalar2=None,
                                        op0=mybir.AluOpType.add)
                # h^T via PE transpose -> feat2 = h @ W2ext
                pt = pst.tile([P, HID], dt.float16, space="PSUM", tag="t")
                nc.tensor.transpose(out=pt[:, 0:P], in_=h16[:, 0:P], identity=ident[:])
                nc.tensor.transpose(out=pt[:, P:HID], in_=h16[:, P:HID], identity=ident[:])
                hT = hpool.tile([P, 2, P], dt.float16, tag="hT")
                nc.vector.tensor_copy(hT[:, 0, :], pt[:, 0:P])
                nc.vector.tensor_copy(hT[:, 1, :], pt[:, P:HID])
                p2 = pst.tile([P, OUT_F + 2], dt.float32, space="PSUM", tag="t")
                nc.tensor.matmul(out=p2[:], lhsT=hT[:, 0, :], rhs=w2e[:, 0, :], start=True, stop=False)
                nc.tensor.matmul(out=p2[:], lhsT=hT[:, 1, :], rhs=w2e[:, 1, :], start=False, stop=True)
                t2row = hpool.tile([P, OUT_F + 1], dt.float16, tag="t2row")
                nc.vector.tensor_copy(t2row[:], p2[:, 0:OUT_F + 1])
                nc.vector.tensor_copy(er2_res[:, b:b + 1], p2[:, OUT_F + 1:OUT_F + 2])
                nc.sync.dma_start(out=t2_shard[b * P:(b + 1) * P, 0:OUT_F + 1], in_=t2row[:])

            # ================= allgather L2 table =================
            if single:
                for c in range(NCORES):
                    nc.sync.dma_start(out=t2_full[c * SHARD:(c + 1) * SHARD, :], in_=t2_shard[:, :])
            else:
                nc.gpsimd.collective_compute(
                    "AllGather", mybir.AluOpType.bypass, replica_groups=groups,
                    ins=[t2_shard[:, :]], outs=[t2_full[:, :]],
                )

            # ================= layer 2 edge loop =================
            for b in range(NBLK):
                J = int(Jb[b])
                g2 = gpool.tile([P, Jmax + 1, ELEM2], dt.float16, tag="g2")
                for (cb, j0, span, col0, ncols, nidx) in calls:
                    if cb != b:
                        continue
                    _pool_gather(
                        nc, g2[:, j0:j0 + span + 1, :], t2_full[SHIFT:, :],
                        idx_sb[:, col0:col0 + ncols], nidx, ELEM2,
                    )
                el2 = g2[:, 0:J, OUT_F:OUT_F + 1]
                er_b = er2_res[:, b:b + 1].unsqueeze(2).to_broadcast([P, J, 1])
                e1 = epool.tile([P, Jmax, 1], dt.float32, tag="f1")
                nc.vector.tensor_tensor(out=e1[:, 0:J, :], in0=el2, in1=er_b, op=mybir.AluOpType.add)
                e2 = epool.tile([P, Jmax, 1], dt.float32, tag="f2")
                nc.vector.tensor_scalar(out=e2[:, 0:J, :], in0=e1[:, 0:J, :], scalar1=NEG_SLOPE,
                                        scalar2=None, op0=mybir.AluOpType.mult)
                nc.vector.tensor_tensor(out=e2[:, 0:J, :], in0=e2[:, 0:J, :], in1=e1[:, 0:J, :],
                                        op=mybir.AluOpType.max)
                ea = eadd_sb[:, int(jb_off[b]):int(jb_off[b]) + J].unsqueeze(2)
                nc.vector.tensor_tensor(out=e2[:, 0:J, :], in0=e2[:, 0:J, :], in1=ea,
                                        op=mybir.AluOpType.add)
                nc.scalar.activation(g2[:, 0:J, OUT_F:OUT_F + 1], e2[:, 0:J, :],
                                     mybir.ActivationFunctionType.Exp, bias=EXP_BIAS)
                den = epool.tile([P, 1], dt.float32, tag="den2")
                nc.vector.tensor_reduce(
                    out=den[:], in_=g2[:, 0:J, OUT_F:OUT_F + 1].rearrange("p j h -> p h j"),
                    axis=mybir.AxisListType.X, op=mybir.AluOpType.add,
                )
                nc.vector.tensor_scalar(out=den[:], in0=den[:], scalar1=1e-30, scalar2=None,
                                        op0=mybir.AluOpType.add)
                rec = epool.tile([P, 1], dt.float32, tag="rec2")
                nc.vector.reciprocal(rec[:], den[:])
                eexp_b = g2[:, 0:J, OUT_F:OUT_F + 1].to_broadcast([P, J, OUT_F])
                nc.vector.tensor_tensor(out=g2[:, 0:J, 0:OUT_F], in0=g2[:, 0:J, 0:OUT_F],
                                        in1=eexp_b, op=mybir.AluOpType.mult)
                pa = ps.tile([P, OUT_F], dt.float32, space="PSUM", tag="acc")
                for j in range(J):
                    nc.tensor.matmul(out=pa[:], lhsT=ident[:], rhs=g2[:, j, 0:OUT_F],
                                     start=(j == 0), stop=(j == J - 1))
                # normalize; single head -> mean over heads is identity
                rstn = hpool.tile([P, OUT_F], dt.float32, tag="rst2")
                nc.vector.tensor_scalar(out=rstn[:], in0=pa[:], scalar1=rec[:, 0:1],
                                        scalar2=None, op0=mybir.AluOpType.mult)
                # log_softmax over the 40 classes
                mx = epool.tile([P, 1], dt.float32, tag="mx2")
                nc.vector.tensor_reduce(out=mx[:], in_=rstn[:], axis=mybir.AxisListType.X,
                                        op=mybir.AluOpType.max)
                sub = hpool.tile([P, OUT_F], dt.float32, tag="sub2")
                nc.vector.tensor_scalar(out=sub[:], in0=rstn[:], scalar1=mx[:, 0:1],
                                        scalar2=None, op0=mybir.AluOpType.subtract)
                ex = hpool.tile([P, OUT_F], dt.float32, tag="ex2")
                sm = epool.tile([P, 1], dt.float32, tag="sm2")
                nc.scalar.activation(ex[:], sub[:], mybir.ActivationFunctionType.Exp,
                                     accum_out=sm[:])
                lg = epool.tile([P, 1], dt.float32, tag="lg2")
                nc.scalar.activation(lg[:], sm[:], mybir.ActivationFunctionType.Ln)
                outt = hpool.tile([P, OUT_F], dt.float32, tag="outt")
                nc.vector.tensor_scalar(out=outt[:], in0=sub[:], scalar1=lg[:, 0:1],
                                        scalar2=None, op0=mybir.AluOpType.subtract)
                nc.sync.dma_start(out=out_d[b * P:(b + 1) * P, :], in_=outt[:])

    nc.compile()
    return nc


_CACHE = {}
_LAST_INMAPS = None


def kernel(features, src, dst, W1, al1, ar1, b1, W2, al2, ar2, b2):
    features = np.asarray(features, dtype=np.float32)
    src = np.asarray(src, dtype=np.int32)
    dst = np.asarray(dst, dtype=np.int32)
    W1 = np.asarray(W1, dtype=np.float32)
    al1 = np.asarray(al1, dtype=np.float32)
    ar1 = np.asarray(ar1, dtype=np.float32)
    W2 = np.asarray(W2, dtype=np.float32)
    al2 = np.asarray(al2, dtype=np.float32)
    ar2 = np.asarray(ar2, dtype=np.float32)
    assert np.all(np.asarray(b1) == 0) and np.all(np.asarray(b2) == 0), \
        "kernel assumes zero biases (reference setup uses zeros)"

    merged, streams, meta = build_plan(src, dst)
    Jmax = merged["Jmax"]

    key = ("nc", merged["idx_tile_cols"], merged["T"], Jmax)
    if key not in _CACHE:
        _CACHE[key] = build_nc(merged, Jmax, reps=int(os.environ.get("GAT_REPS", "1")))
    nc = _CACHE[key]

    # weight prep
    almat = np.zeros((HID, H1), dtype=np.float32)
    armat = np.zeros((HID, H1), dtype=np.float32)
    for h in range(H1):
        almat[h * D1:(h + 1) * D1, h] = al1[h]
        armat[h * D1:(h + 1) * D1, h] = ar1[h]
    w1e = np.concatenate([W1, W1 @ almat, W1 @ armat], axis=1).astype(np.float16)  # [256, 264]
    w2e = np.concatenate([W2, W2 @ al2[0][:, None], W2 @ ar2[0][:, None]], axis=1).astype(np.float16)  # [256, 42]

    order = meta["order"]
    in_maps = []
    for c in range(NCORES):
        # x^T shard: columns = core's node positions (dummies -> 0)
        xT = np.zeros((IN_F, SHARD), dtype=np.float16)
        for b in range(NBLK):
            g = b * NCORES + c          # global block id
            lo = g * P
            hi = min(lo + P, N)
            if hi > lo:
                nodes = order[lo:hi]
                xT[:, b * P:b * P + (hi - lo)] = features[nodes].T.astype(np.float16)
        in_maps.append(dict(
            xT=xT, w1e=w1e, w2e=w2e,
            idx=streams[c]["idx_tile"], eadd=streams[c]["eadd_tile"],
        ))

    global _LAST_INMAPS
    _LAST_INMAPS = in_maps
    res = run_bass_kernel_spmd(nc, in_maps, list(range(NCORES)))

    out = np.zeros((N, OUT_F), dtype=np.float32)
    for c in range(NCORES):
        lo_out = res.results[c]["logits"]       # [SHARD, 40]
        for b in range(NBLK):
            g = b * NCORES + c
            lo = g * P
            hi = min(lo + P, N)
            if hi > lo:
                out[order[lo:hi]] = lo_out[b * P:b * P + (hi - lo)]
    return out

